# revision 1
# baseline (speedup 1.0000x reference)
"""ColBERT late-interaction scoring kernel for Trainium2 (Bass/Tile).

score_b = sum_q max_k (2*Q@D^T - ||q||^2 - ||d||^2)
        = sum_q max_k (2*qd[q,k] - d_sq[k])  -  ||Q_b||_F^2

Sharding: data-parallel over batch B=128 across 8 NeuronCores (16 each).

Per-core per-batch device pipeline:
  1. SWDGE cast-DMA loads Q,D (f32 DRAM -> bf16 SBUF, natural layout)
  2. HWDGE xbar-transpose DMAs build QT,DT [d=128, L=512] bf16
  3. GPSIMD computes SQ = DT*DT
  4. PE matmul (lhsT = -0.5*ones) broadcasts -0.5*d_sq into a PSUM bank
  5. PE matmuls (lhsT = QT tile) produce qd [128q, 512k] PSUM tiles
  6. DVE tensor_tensor_reduce fuses (qd - 0.5*d_sq)*2 + max_k per tile
  7. DVE ttr accumulates per-partition ||Q||^2 partials
  Endgame: small reduce + ones-matmul partition sum -> [16] scores.
"""

import numpy as np

B, LQ, LD, D = 128, 512, 512, 128
N_CORES = 8
BPC = B // N_CORES  # batches per core
NT = LQ // 128  # q tiles per batch

_compiled = {}


def _cast_gpsimd(nc, dst, src):
    nc.gpsimd.tensor_copy(dst, src)


def _cast_act(nc, dst, src):
    nc.scalar.copy(dst, src)


def _cast_dve(nc, dst, src):
    nc.vector.tensor_copy(dst, src)


_CAST_Q = _cast_gpsimd
_CAST_D = _cast_gpsimd


def _split_multi_waits(nc):
    """This container's walrus accepts only ONE sem-wait per instruction
    (setupSyncWait: 'Too many sync wait commands'). Tile's wait assignment
    emits multi-wait instructions, so split: every extra wait moves onto a
    dedicated NoOp inserted just before the instruction on the same engine.
    Engine program order makes this semantically identical."""
    import concourse.mybir as mybir

    for f in nc.m.functions:
        for blk in f.blocks:
            il = blk.instructions
            i = 0
            while i < len(il):
                inst = il[i]
                si = inst.sync_info
                waits = list(si.on_wait) if si and si.on_wait else []
                if len(waits) > 1:
                    for w in waits[:-1]:
                        nop = mybir.InstNoOp(
                            name=nc.get_next_instruction_name(), ins=[], outs=[]
                        )
                        nop.engine = inst.engine
                        nop.sync_info = mybir.SyncInfo(on_wait=[w], on_update=[])
                        il.insert(i, nop)
                        i += 1
                    inst.sync_info = mybir.SyncInfo(
                        on_wait=[waits[-1]], on_update=si.on_update
                    )
                i += 1


def _build(reps: int = 1):
    import concourse.bass as bass
    import concourse.mybir as mybir
    import concourse.tile as tile
    from concourse.bass import ts

    nc = bass.Bass()
    f32 = mybir.dt.float32
    bf16 = mybir.dt.bfloat16

    qe = nc.dram_tensor("qe", [BPC, LQ, D], f32, kind="ExternalInput")
    de = nc.dram_tensor("de", [BPC, LD, D], f32, kind="ExternalInput")
    out = nc.dram_tensor("out", [1, BPC], f32, kind="ExternalOutput")

    with tile.TileContext(nc) as tc:
        with (
            tc.tile_pool(name="consts", bufs=1) as cpool,
            tc.tile_pool(name="work", bufs=3) as wpool,
            tc.tile_pool(name="acc", bufs=1) as apool,
            tc.tile_pool(name="ps", bufs=4, space="PSUM") as pspool,
        ):
            neg_half = cpool.tile([128, 128], bf16)
            nc.gpsimd.memset(neg_half, -0.5)
            ones_col = cpool.tile([128, 1], f32)
            nc.gpsimd.memset(ones_col, 1.0)

            # rowmax accumulator [128, BPC*NT] and qsq accumulator [128, BPC]
            MX = apool.tile([128, BPC * NT], f32)
            QS = apool.tile([128, BPC], f32)

            GRP = 4  # batches per load/cast group
            NG = BPC // GRP
            for rep in range(reps):
                # Phase 1: HWDGE f32 loads, 4 batches per DMA (SWDGE
                # cast-DMAs measured ~2ms each here — pathological). Layout
                # q = NT*p + t: partition p covers NT consecutive DRAM rows =
                # one contiguous run. Tile t then holds q's {NT*p + t}, a
                # harmless reordering (scores sum over all q).
                qbs, dbs = [], []
                for g in range(NG):
                    qf = wpool.tile([128, GRP, NT, 128], f32, tag="qf")
                    df = wpool.tile([128, GRP, NT, 128], f32, tag="df")
                    nc.sync.dma_start(
                        qf, qe[g * GRP : (g + 1) * GRP].rearrange("b (p t) d -> p b t d", t=NT)
                    )
                    nc.sync.dma_start(
                        df, de[g * GRP : (g + 1) * GRP].rearrange("b (p t) d -> p b t d", t=NT)
                    )
                    # Phase 2: cast f32 -> bf16 (gpsimd; 1-input ~line rate).
                    # bf16 tiles get a fresh slot per group: WAR waits would
                    # land on XPOSE DMAs, which allow a single sem-wait.
                    qb = wpool.tile([128, GRP, NT, 128], bf16, tag="qb", bufs=NG)
                    db = wpool.tile([128, GRP, NT, 128], bf16, tag="db", bufs=NG)
                    _CAST_Q(nc, qb, qf)
                    _CAST_D(nc, db, df)
                    qbs.append(qb)
                    dbs.append(db)

                # Phase 3: xbar transposes -> QT/DT [d, L] per batch
                QTs, DTs = [], []
                for b in range(BPC):
                    g, i = b // GRP, b % GRP
                    QT = wpool.tile([128, LQ], bf16, tag="QT", bufs=BPC)
                    DT = wpool.tile([128, LD], bf16, tag="DT", bufs=BPC)
                    for t in range(NT):
                        nc.sync.dma_start_transpose(QT[:, ts(t, 128)], qbs[g][:, i, t, :])
                        nc.sync.dma_start_transpose(DT[:, ts(t, 128)], dbs[g][:, i, t, :])
                    QTs.append(QT)
                    DTs.append(DT)

                # Phase 4: SQ = DT*DT and qsq accumulation (ScalarE)
                SQs = []
                for b in range(BPC):
                    SQ = wpool.tile([128, LD], bf16, tag="SQ", bufs=BPC)
                    nc.scalar.activation(SQ, DTs[b], mybir.ActivationFunctionType.Square)
                    SQs.append(SQ)
                    g, i = b // GRP, b % GRP
                    junkb = wpool.tile([128, NT, 128], bf16, tag="junkb")
                    nc.scalar.activation(
                        junkb,
                        qbs[g][:, i],
                        mybir.ActivationFunctionType.Square,
                        accum_out=QS[:, b : b + 1],
                    )

                # Phase 5: per batch, 4 accumulation groups (bias bcast + qd)
                # split across two 2-bank psum tiles (bufs=4), each closed by
                # a fused rowmax reduce. Finer PSUM granularity lets PE start
                # the next half-batch while DVE reduces the previous one —
                # measured ~19% faster than one 4-bank tile x 2 bufs.
                # (Measured alternatives on this runtime: single matmuls +
                # DVE bias-add in PSUM serialize PE<->DVE at tile granularity
                # and are ~2x slower overall than the accumulation groups.)
                for b in range(BPC):
                  for h in range(2):
                    pst = pspool.tile([128, NT // 2, LD], f32, tag="pst")
                    for t2 in range(NT // 2):
                        t = h * (NT // 2) + t2
                        nc.tensor.matmul(
                            pst[:, t2, :], lhsT=neg_half, rhs=SQs[b],
                            start=True, stop=False,
                        )
                        nc.tensor.matmul(
                            pst[:, t2, :],
                            lhsT=QTs[b][:, ts(t, 128)],
                            rhs=DTs[b],
                            start=False,
                            stop=True,
                        )
                    nc.vector.reduce_max(
                        MX[:, b * NT + h * (NT // 2) : b * NT + (h + 1) * (NT // 2)],
                        pst, axis=mybir.AxisListType.X
                    )

            # Endgame: SC[p, b] = 2 * sum_t MX[p, b*NT+t] - QS[p, b]
            msum = apool.tile([128, BPC], f32)
            nc.vector.reduce_sum(
                msum, MX.rearrange("p (b t) -> p b t", t=NT), axis=mybir.AxisListType.X
            )
            msum2 = apool.tile([128, BPC], f32)
            nc.vector.tensor_scalar_mul(msum2, msum, 2.0)
            sc = apool.tile([128, BPC], f32)
            nc.vector.tensor_tensor(sc, msum2, QS, op=mybir.AluOpType.subtract)

            # partition sum via ones-matmul -> [1, BPC] (reuses a pst slot)
            ps_s = pspool.tile([1, BPC], f32, tag="pst")
            nc.tensor.matmul(ps_s, lhsT=ones_col, rhs=sc, start=True, stop=True)
            score = apool.tile([1, BPC], f32)
            nc.vector.tensor_copy(score, ps_s)
            nc.sync.dma_start(out[:, :], score)

    _split_multi_waits(nc)
    return nc


def kernel(query_embedding: np.ndarray, document_embedding: np.ndarray) -> np.ndarray:
    from concourse.bass_utils import run_bass_kernel_spmd

    if "nc" not in _compiled:
        _compiled["nc"] = _build()
    nc = _compiled["nc"]

    qe = np.ascontiguousarray(query_embedding, dtype=np.float32)
    de = np.ascontiguousarray(document_embedding, dtype=np.float32)
    in_maps = [
        {"qe": qe[c * BPC : (c + 1) * BPC], "de": de[c * BPC : (c + 1) * BPC]}
        for c in range(N_CORES)
    ]
    res = run_bass_kernel_spmd(nc, in_maps, core_ids=list(range(N_CORES)))
    return np.concatenate(
        [res.results[c]["out"].reshape(BPC) for c in range(N_CORES)]
    ).astype(np.float32)

